# revision 66
# baseline (speedup 1.0000x reference)
"""DifferentialWindowAttention TRN2 kernel — 8-core SPMD, data-parallel over windows.

Layout: channel-transposed (CT) activations [C(part), tokens(free)].
 - Projections as CT GEMMs; the heavy ones (dino 1024-dim, q, kg, ks, vtok) run in
   fp8e4m3 with DoubleRow perf mode (two 128-row contraction blocks per instruction
   at 0.5 cyc/col). Power-of-2 weight scaling keeps fp8 in range; the psum->sbuf
   copy descales (scalar activation scale) and adds biases (per-partition bias AP).
 - Attention: S^T[m,q] = (kT-slice as lhsT) @ (qT-slice as rhs); softmax without
   max-subtraction (logits are tiny): U = exp(S^T) * exp_rpb.
   Denominators via PE band-select ones-matmuls -> [4*32-band, (wpair,q)] broadcast.
 - AV: lhsT = token-major V slices, rhs = U slices -> Z^T in CT layout.
 - sub-LN in CT: stats via PE ones-matmuls; rstd = exp(-0.5*ln(var+eps)) on the
   scalar engine (keeps every activation in the natural_log_exp table set -> no
   ACT table reloads); gamma folded into w_proj rows, beta folded into the output
   bias column.
 - DRAM output is [C, T] in a fixed token permutation inverted on host.
"""
import math
import numpy as np
import ml_dtypes

import concourse.bass as bass
import concourse.tile as tile
from concourse import mybir
from concourse.bass_utils import run_bass_kernel_spmd

BF16 = mybir.dt.bfloat16
F32 = mybir.dt.float32
F32R = mybir.dt.float32r
FP8 = mybir.dt.float8e4
AF = mybir.ActivationFunctionType
ALU = mybir.AluOpType
DR = mybir.MatmulPerfMode.DoubleRow

B, N, C, H, D, WIN = 1024, 64, 256, 8, 32, 8
NCORES = 8
BW = B // NCORES            # windows per core
LAMBDA_INIT = 0.8 - 0.6 * math.exp(-0.3 * 1)
EPS = 1e-5

SQ = 256.0   # fp8 scale for wq / wdino (+ident)
SK = 64.0    # fp8 scale for wkvg / wkvs / wkvsn / w2g

_CACHE = {}


def _legalize_waits(nc, max_waits=1):
    """Old walrus in this container allows one sync-wait per instruction;
    hoist extras into standalone EventSemaphore instructions just before."""
    ctr = 0
    for f in nc.m.functions:
        for bb in f.blocks:
            new = []
            for inst in bb.instructions:
                si = inst.sync_info
                if si is not None and si.on_wait and len(si.on_wait) > max_waits:
                    waits = list(si.on_wait)
                    for w in waits[max_waits:]:
                        ctr += 1
                        ev = mybir.InstEventSemaphore(
                            name=f"waitfix_{ctr}", ins=[], outs=[],
                            engine=inst.engine,
                            sync_info=mybir.SyncInfo(on_wait=[w], on_update=[]))
                        new.append(ev)
                    inst.sync_info = mybir.SyncInfo(on_wait=waits[:max_waits],
                                                    on_update=list(si.on_update or []))
                new.append(inst)
            bb.instructions = new
    return ctr


def build_bass(T, tap=None, legalize=True):
    NG = T // 512
    TG = 512
    nc = bass.Bass()
    xT = nc.declare_dram_parameter("xT", [C, T], BF16, isOutput=False)
    dinoT = nc.declare_dram_parameter("dinoT", [1024, T], BF16, isOutput=False)
    pfT = nc.declare_dram_parameter("pfT", [4, T], F32R, isOutput=False)
    bias6_d = nc.declare_dram_parameter("bias6", [C, 6], F32, isOutput=False)
    wq_a = nc.declare_dram_parameter("wq_a", [C, C], BF16, isOutput=False)
    wkvg_a = nc.declare_dram_parameter("wkvg_a", [C, 2 * C], BF16, isOutput=False)
    wdino_a = nc.declare_dram_parameter("wdino_a", [1024, C], BF16, isOutput=False)
    wkvs_a = nc.declare_dram_parameter("wkvs_a", [C, 2 * C], BF16, isOutput=False)
    wkvsn_a = nc.declare_dram_parameter("wkvsn_a", [C, C], BF16, isOutput=False)
    w2g_a = nc.declare_dram_parameter("w2g_a", [4, 2 * C], F32R, isOutput=False)
    wproj_a = nc.declare_dram_parameter("wproj_a", [C, C], F32R, isOutput=False)
    ident_d = nc.declare_dram_parameter("ident", [128, 128], BF16, isOutput=False)
    band_d = nc.declare_dram_parameter("band", [2, 128, 32], BF16, isOutput=False)
    rpb_d = nc.declare_dram_parameter("exp_rpb", [128, H * 256], BF16, isOutput=False)
    crow_f_d = nc.declare_dram_parameter("crow_f", [1, 384], F32R, isOutput=False)
    ccol_f_d = nc.declare_dram_parameter("ccol_f", [128, 1], F32R, isOutput=False)
    ceps_d = nc.declare_dram_parameter("ceps", [1, 1], F32, isOutput=False)
    outT = nc.declare_dram_parameter("outT", [C, T], F32, isOutput=True)

    import contextlib
    with tile.TileContext(nc) as tc, contextlib.ExitStack() as ctx:
        singles = ctx.enter_context(tc.tile_pool(name="singles", bufs=1))
        inp = ctx.enter_context(tc.tile_pool(name="inp", bufs=2))
        acts = ctx.enter_context(tc.tile_pool(name="acts", bufs=2))
        attn = ctx.enter_context(tc.tile_pool(name="attn", bufs=2))
        outs = ctx.enter_context(tc.tile_pool(name="outs", bufs=2))
        psum = ctx.enter_context(tc.tile_pool(name="psum", bufs=1, space="PSUM"))

        # ---------------- constants ----------------
        _cn = [0]

        def cload(src, shape, dt):
            _cn[0] += 1
            t = singles.tile(shape, dt, tag=f"c{_cn[0]}", name=f"c{_cn[0]}")
            nc.sync.dma_start(out=t, in_=src)
            return t

        b6 = [cload(bias6_d[0:128, :], [128, 6], F32),
              cload(bias6_d[128:256, :], [128, 6], F32)]
        wq_t = [cload(wq_a[0:128, :], [128, C], BF16),
                cload(wq_a[128:256, :], [128, C], BF16)]
        wkvg_t = [cload(wkvg_a[0:128, :], [128, 2 * C], BF16),
                  cload(wkvg_a[128:256, :], [128, 2 * C], BF16)]
        wdino_t = [cload(wdino_a[k * 128:(k + 1) * 128, :], [128, C], BF16) for k in range(8)]
        wkvs_t = [cload(wkvs_a[0:128, :], [128, 2 * C], BF16),
                  cload(wkvs_a[128:256, :], [128, 2 * C], BF16)]
        wkvsn_t = [cload(wkvsn_a[0:128, :], [128, C], BF16),
                   cload(wkvsn_a[128:256, :], [128, C], BF16)]
        w2g_t = cload(w2g_a[:, :], [4, 2 * C], F32R)
        wproj_t = [cload(wproj_a[0:128, :], [128, C], F32R),
                   cload(wproj_a[128:256, :], [128, C], F32R)]
        ident_t = cload(ident_d[:, :], [128, 128], BF16)
        band_t = [cload(band_d[p, :, :], [128, 32], BF16) for p in range(2)]
        rpb_t = [cload(rpb_d[:, h * 256:(h + 1) * 256], [128, 256], BF16) for h in range(H)]
        crow = cload(crow_f_d[:, :], [1, 384], F32R)
        ones_bc = crow[:, 256:384]
        oneC_col = cload(ccol_f_d[:, :], [128, 1], F32R)
        eps_t = cload(ceps_d[:, :], [1, 1], F32)

        MM = nc.tensor.matmul
        pg = [0]

        def gemm_ps():
            t = psum.tile([128, TG], F32, tag=f"pga{pg[0] % 2}", name=f"pga{pg[0] % 2}")
            pg[0] += 1
            return t

        def ln_ps():
            return gemm_ps()

        def make_half_alloc(tag, nb=2):
            """Yield [128,256] psum slices, packing pairs into [128,512] banks, 2 banks cycling."""
            state = {"i": 0, "cur": None}

            def alloc():
                i = state["i"]
                if i % 2 == 0:
                    state["cur"] = psum.tile([128, 512], F32, tag=f"{tag}{(i // 2) % nb}",
                                             name=f"{tag}{(i // 2) % nb}")
                state["i"] += 1
                return state["cur"][:, (i % 2) * 256:(i % 2) * 256 + 256]
            return alloc

        # First loop iteration computes wrong on this HW (startup race in the
        # scalar/PE pipeline); re-run group 0 at the end to overwrite its output.
        for g in list(range(NG)) + [0]:
            sl = slice(g * TG, (g + 1) * TG)
            # ---------------- loads ----------------
            xt = [inp.tile([128, TG], BF16, tag=f"xt{i}", name=f"xt{i}") for i in range(2)]
            nc.sync.dma_start(out=xt[0], in_=xT[0:128, sl])
            nc.sync.dma_start(out=xt[1], in_=xT[128:256, sl])
            dt_ = [inp.tile([128, TG], BF16, tag=f"dt{k}", name=f"dt{k}") for k in range(8)]
            for k in range(8):
                nc.sync.dma_start(out=dt_[k], in_=dinoT[k * 128:(k + 1) * 128, sl])
            pft = inp.tile([4, TG], F32R, tag="pft", name="pft")
            nc.sync.dma_start(out=pft, in_=pfT[:, sl])

            # ---------------- q GEMM (fp8 DR) ----------------
            q_sb = [acts.tile([128, TG], BF16, tag=f"q{m}", name=f"q{m}") for m in range(2)]
            for m in range(2):
                ps = gemm_ps()
                c0, c1 = m * 128, (m + 1) * 128
                MM(ps, wq_t[0][:, c0:c1], xt[0], start=True, stop=False)
                MM(ps, wq_t[1][:, c0:c1], xt[1], start=False, stop=True)
                nc.scalar.activation(out=q_sb[m], in_=ps, func=AF.Identity,
                                     bias=b6[m][:, 0:1])

            if tap == "q":
                for m in range(2):
                    nc.gpsimd.dma_start(out=outT[m * 128:(m + 1) * 128, sl], in_=q_sb[m])
                continue
            # ---------------- sem_enh GEMM (fp8 DR dino + scaled-identity x) --------
            se_sb = [acts.tile([128, TG], BF16, tag=f"se{m}", name=f"se{m}") for m in range(2)]
            for m in range(2):
                ps = gemm_ps()
                c0, c1 = m * 128, (m + 1) * 128
                for k in range(8):
                    MM(ps, wdino_t[k][:, c0:c1], dt_[k], start=(k == 0), stop=False)
                MM(ps, ident_t, xt[m], start=False, stop=True)
                nc.scalar.activation(out=se_sb[m], in_=ps, func=AF.Identity,
                                     bias=b6[m][:, 2:3])

            if tap == "se":
                for m in range(2):
                    nc.gpsimd.dma_start(out=outT[m * 128:(m + 1) * 128, sl], in_=se_sb[m])
                continue
            # ---------------- k_geo / k_sem GEMMs (fp8 DR) ----------------
            kg_sb = [acts.tile([128, TG], BF16, tag=f"kg{m}", name=f"kg{m}") for m in range(2)]
            ks_sb = [acts.tile([128, TG], BF16, tag=f"ks{m}", name=f"ks{m}") for m in range(2)]
            for m in range(2):
                c0, c1 = m * 128, (m + 1) * 128
                ps = gemm_ps()
                MM(ps, wkvg_t[0][:, c0:c1], xt[0], start=True, stop=False)
                MM(ps, wkvg_t[1][:, c0:c1], xt[1], start=False, stop=False)
                MM(ps, w2g_t[:, c0:c1], pft, start=False, stop=True)
                nc.scalar.copy(out=kg_sb[m], in_=ps)
            for m in range(2):
                c0, c1 = m * 128, (m + 1) * 128
                ps = gemm_ps()
                MM(ps, wkvs_t[0][:, c0:c1], se_sb[0], start=True, stop=False)
                MM(ps, wkvs_t[1][:, c0:c1], se_sb[1], start=False, stop=True)
                nc.scalar.activation(out=ks_sb[m], in_=ps, func=AF.Identity,
                                     bias=b6[m][:, 1:2])

            if tap in ("kg", "ks"):
                tt_ = {"kg": kg_sb, "ks": ks_sb}[tap]
                for m in range(2):
                    nc.gpsimd.dma_start(out=outT[m * 128:(m + 1) * 128, sl], in_=tt_[m])
                continue
            # ---------------- token-major V GEMMs (fp8 DR) ----------------
            # vtok[kind][c] : [128 tok = 2 windows, 256 = 8h x 32d], c = token chunk
            vtok = {"vm": [], "vs": []}
            for c in range(4):
                t0c = c * 128
                vps = psum.tile([128, 512], F32, tag=f"pu{c % 2}", name=f"pu{c % 2}")
                ps = vps[:, 0:256]
                MM(ps, xt[0][:, t0c:t0c + 128], wkvg_t[0][:, 256:512], start=True, stop=False)
                MM(ps, xt[1][:, t0c:t0c + 128], wkvg_t[1][:, 256:512], start=False, stop=False)
                MM(ps, pft[:, t0c:t0c + 128], w2g_t[:, 256:512], start=False, stop=False)
                MM(ps, se_sb[0][:, t0c:t0c + 128], wkvsn_t[0], start=False, stop=False)
                MM(ps, se_sb[1][:, t0c:t0c + 128], wkvsn_t[1], start=False, stop=True)
                vt = attn.tile([128, 256], BF16, tag=f"vt_vm{c}", name=f"vt_vm{c}")
                nc.scalar.copy(out=vt, in_=ps)
                vtok["vm"].append(vt)
                ps2 = vps[:, 256:512]
                MM(ps2, se_sb[0][:, t0c:t0c + 128], wkvs_t[0][:, 256:512], start=True, stop=False)
                MM(ps2, se_sb[1][:, t0c:t0c + 128], wkvs_t[1][:, 256:512], start=False, stop=True)
                vt2 = attn.tile([128, 256], BF16, tag=f"vt_vs{c}", name=f"vt_vs{c}")
                nc.scalar.copy(out=vt2, in_=ps2)
                vtok["vs"].append(vt2)

            # ---------------- attention ----------------
            # U tiles per (branch, head): [128 = m + 64*(w%2), 256 = (w//2)*64 + q]
            Ur = {}
            qk_ps = make_half_alloc("pu")
            for br, ktiles in (("g", kg_sb), ("s", ks_sb)):
                for h in range(H):
                    kt = ktiles[h // 4]
                    qt = q_sb[h // 4]
                    r0 = (h % 4) * 32
                    ps = qk_ps()
                    for w in range(8):
                        MM(ps[64 * (w % 2):64 * (w % 2) + 64, (w // 2) * 64:(w // 2) * 64 + 64],
                           kt[r0:r0 + 32, w * 64:(w + 1) * 64],
                           qt[r0:r0 + 32, w * 64:(w + 1) * 64],
                           start=True, stop=True,
                           tile_position=(r0, 64 * (w % 2)))
                    ue = attn.tile([128, 256], BF16, tag=f"ue_{br}{h}", name=f"ue_{br}{h}")
                    nc.scalar.activation(out=ue, in_=ps, func=AF.Exp)
                    ur = attn.tile([128, 256], BF16, tag=f"ur_{br}{h}", name=f"ur_{br}{h}")
                    nc.gpsimd.tensor_tensor(out=ur, in0=ue, in1=rpb_t[h], op=ALU.mult)
                    Ur[(br, h)] = ur

            if tap == "U":
                nc.gpsimd.dma_start(out=outT[0:128, slice(g*512, g*512+256)], in_=Ur[("g", 0)])
                nc.gpsimd.dma_start(out=outT[128:256, slice(g*512, g*512+256)], in_=Ur[("g", 1)])
                nc.gpsimd.dma_start(out=outT[0:128, slice(g*512+256, g*512+512)], in_=Ur[("s", 0)])
                nc.gpsimd.dma_start(out=outT[128:256, slice(g*512+256, g*512+512)], in_=Ur[("s", 1)])
                continue
            if tap == "vt":
                for c in range(2):
                    nc.gpsimd.dma_start(out=outT[0:128, slice(g*512+c*256, g*512+(c+1)*256)], in_=vtok["vm"][c])
                    nc.gpsimd.dma_start(out=outT[128:256, slice(g*512+c*256, g*512+(c+1)*256)], in_=vtok["vs"][c])
                continue
            # AV with interleaved denominator band-matmuls: the band MMs for each
            # (q2, par) issue right before its AV block so the reciprocal-gated
            # prs recycling does not head-of-line-block the in-order PE queue.
            rs = {}
            rs_ps = make_half_alloc("prs", nb=3)
            opre = {}
            for q2 in range(2):
                for par in range(2):
                    for br in ("g", "s"):
                        ps = rs_ps()
                        for hp in range(4):
                            MM(ps[hp * 32:(hp + 1) * 32, :],
                               band_t[par], Ur[(br, 4 * q2 + hp)],
                               start=True, stop=True,
                               tile_position=(0, hp * 32))
                        r = attn.tile([128, 256], F32, tag=f"rs_{br}{q2}{par}", name=f"rs_{br}{q2}{par}")
                        nc.vector.reciprocal(out=r, in_=ps)
                        rs[(br, q2, par)] = r
                    zt = psum.tile([128, 512], F32, tag="pz0", name="pz0")
                    zps = {}
                    for br, kind in (("g", "vm"), ("s", "vs")):
                        ps = zt[:, 0:256] if br == "g" else zt[:, 256:512]
                        for hp in range(4):
                            h = 4 * q2 + hp
                            for wp in range(4):
                                MM(ps[hp * 32:(hp + 1) * 32, wp * 64:(wp + 1) * 64],
                                   vtok[kind][wp][64 * par:64 * par + 64, h * 32:(h + 1) * 32],
                                   Ur[(br, h)][64 * par:64 * par + 64, wp * 64:(wp + 1) * 64],
                                   start=True, stop=True,
                                   tile_position=(64 * par, hp * 32))
                        zps[br] = ps
                    t1 = outs.tile([128, 256], F32, tag="t1", name="t1")
                    t2 = outs.tile([128, 256], F32, tag="t2", name="t2")
                    nc.vector.tensor_tensor(out=t1, in0=zps["g"], in1=rs[("g", q2, par)], op=ALU.mult)
                    nc.vector.tensor_tensor(out=t2, in0=zps["s"], in1=rs[("s", q2, par)], op=ALU.mult)
                    # osq: [:, 0:256] = t1 + t2 + v-bias col, [:, 256:512] = square (stats)
                    op_ = outs.tile([128, 512], F32R, tag=f"opre{q2}{par}", name=f"opre{q2}{par}")
                    nc.vector.scalar_tensor_tensor(out=op_[:, 0:256], in0=t1,
                                                   scalar=b6[q2][:, 3:4], in1=t2,
                                                   op0=ALU.add, op1=ALU.add)
                    nc.gpsimd.tensor_tensor(out=op_[:, 256:512], in0=op_[:, 0:256],
                                            in1=op_[:, 0:256], op=ALU.mult)
                    opre[(q2, par)] = op_

            if tap == "rs":
                for i, k_ in enumerate([("g",0,0),("g",0,1),("g",1,0),("g",1,1)][:2]):
                    nc.sync.dma_start(out=outT[i*128:(i+1)*128, slice(g*512, g*512+256)], in_=rs[k_])
                continue
            if tap == "opre":
                for q2 in range(2):
                    for par in range(2):
                        nc.sync.dma_start(out=outT[q2*128:(q2+1)*128, slice(g*512+par*256, g*512+(par+1)*256)], in_=opre[(q2, par)][:, 0:256])
                continue
            # ---------------- sub-LN (CT) + final projection ----------------
            # gamma folded into wproj rows; beta folded into output bias col.
            for par in range(2):
                stb = ln_ps()
                st = stb[0:1, :]
                MM(st, oneC_col, opre[(0, par)], start=True, stop=False)
                MM(st, oneC_col, opre[(1, par)], start=False, stop=True)
                stmu = outs.tile([1, 256], F32R, tag="stmu", name="stmu")
                nc.scalar.copy(out=stmu, in_=st[0:1, 0:256])
                stsq = outs.tile([1, 256], F32, tag="stsq", name="stsq")
                nc.scalar.copy(out=stsq, in_=st[0:1, 256:512])
                musq = outs.tile([1, 256], F32, tag="musq", name="musq")
                nc.vector.tensor_tensor(out=musq, in0=stmu, in1=stmu, op=ALU.mult)
                var = outs.tile([1, 256], F32, tag="var", name="var")
                nc.vector.tensor_tensor(out=var, in0=stsq, in1=musq, op=ALU.subtract)
                # rstd = exp(-0.5 * ln(var + eps)) — scalar engine, same ACT table set
                lnv = outs.tile([1, 256], F32, tag="lnv", name="lnv")
                nc.scalar.activation(out=lnv, in_=var, func=AF.Ln, bias=eps_t)
                rstd = outs.tile([1, 256], F32R, tag="rstd", name="rstd")
                nc.scalar.activation(out=rstd, in_=lnv, func=AF.Exp, scale=-0.5)
                bc = ln_ps()
                MM(bc[:, 0:256], ones_bc, stmu, start=True, stop=True)
                MM(bc[:, 256:512], ones_bc, rstd, start=True, stop=True)
                ln = [outs.tile([128, 256], F32R, tag=f"ln{q2}", name=f"ln{q2}") for q2 in range(2)]
                for q2 in range(2):
                    d1 = outs.tile([128, 256], F32, tag="d1", name="d1")
                    nc.vector.tensor_tensor(out=d1, in0=opre[(q2, par)][:, 0:256], in1=bc[:, 0:256], op=ALU.subtract)
                    nc.vector.tensor_tensor(out=ln[q2], in0=d1, in1=bc[:, 256:512], op=ALU.mult)
                pj = ln_ps()
                for m in range(2):
                    c0, c1 = m * 128, (m + 1) * 128
                    ps = pj[:, m * 256:(m + 1) * 256]
                    MM(ps, wproj_t[0][:, c0:c1], ln[0], start=True, stop=False)
                    MM(ps, wproj_t[1][:, c0:c1], ln[1], start=False, stop=True)
                    of = outs.tile([128, 256], F32, tag=f"of{m}", name=f"of{m}")
                    nc.scalar.activation(out=of, in_=ps, func=AF.Identity,
                                         bias=b6[m][:, 4:5])
                    nc.sync.dma_start(out=outT[c0:c1, g * TG + par * 256: g * TG + (par + 1) * 256],
                                      in_=of)
    if legalize:
        _legalize_waits(nc)
    return nc


# ====================== host side ======================

def _dr_pack(w, scale):
    """[256, M] f32 -> [128, 2, M] fp8 DoubleRow lhsT (k-blocks side by side)."""
    f8 = ml_dtypes.float8_e4m3fn
    w = w * scale
    return np.stack([w[0:128, :], w[128:256, :]], 1).astype(f8)


def _prep_consts(inputs, lam):
    f = np.float32
    f8 = ml_dtypes.float8_e4m3fn
    wq = inputs["wq"].astype(f) * (D ** -0.5)
    bq = inputs["bq"].astype(f) * (D ** -0.5)
    wkv_geo = inputs["wkv_geo"].astype(f)
    gw = float(inputs["geo_weight"])
    sw = float(inputs["sem_weight"])
    w2g = gw * (inputs["w_geo_proj"].astype(f) @ wkv_geo)             # [3, 512]
    b2g = inputs["bkv_geo"].astype(f) + gw * (inputs["b_geo_proj"].astype(f) @ wkv_geo)
    w2g_a = np.concatenate([w2g, b2g[None, :]], 0)                    # [4, 512]
    wdino = sw * inputs["w_dino_proj"].astype(f)                      # [1024, 256]
    bdino = sw * inputs["b_dino_proj"].astype(f)
    wkv_sem = inputs["wkv_sem"].astype(f)
    bkv_sem = inputs["bkv_sem"].astype(f)
    sc = f(1.0 - LAMBDA_INIT)
    gamma_s = inputs["ln_gamma"].astype(f) * sc
    beta_s = inputs["ln_beta"].astype(f) * sc
    w_proj = inputs["w_proj"].astype(f)
    wproj_a = gamma_s[:, None] * w_proj                               # gamma fold
    bias6 = np.stack([bq, bkv_sem[0:256], bdino,
                      (1.0 - lam) * bkv_sem[256:512],
                      inputs["b_proj"].astype(f) + beta_s @ w_proj,
                      np.zeros(C, f)], 1)                             # [256, 6]
    wkvsn_a = (-lam) * wkv_sem[:, 256:512]           # [256, 256]
    # exp(rpb) transposed, tiled [128, H*256]
    rpb = inputs["rpb_table"].astype(f)[np.asarray(inputs["rp_index"]).reshape(-1)]
    rpb = rpb.reshape(N, N, H)                                        # [n(q), m, H]
    ex = np.exp(rpb.transpose(2, 1, 0))                               # [H, m, q]
    rpb_tiles = np.zeros((128, H * 256), f)
    for h in range(H):
        blk = np.tile(ex[h], (2, 4)).reshape(128, 256)                # [m+64wp, wpair*64+q]
        rpb_tiles[:, h * 256:(h + 1) * 256] = blk
    ident = np.eye(128, dtype=f)
    band = np.zeros((2, 128, 32), f)
    band[0, 0:64, :] = 1.0
    band[1, 64:128, :] = 1.0
    bf = ml_dtypes.bfloat16
    return {
        "bias6": bias6, "wq_a": wq.astype(bf), "wkvg_a": wkv_geo.astype(bf),
        "wdino_a": wdino.astype(bf), "wkvs_a": wkv_sem.astype(bf),
        "wkvsn_a": wkvsn_a.astype(bf),
        "w2g_a": w2g_a, "wproj_a": wproj_a,
        "ident": ident.astype(bf), "band": band.astype(bf),
        "exp_rpb": rpb_tiles.astype(bf),
        "crow_f": np.ones((1, 384), f),
        "ccol_f": np.full((128, 1), 1.0 / C, f), "ceps": np.full((1, 1), EPS, f),
    }


def _tok_perm(T):
    # device column for linear token t (within a core)
    t = np.arange(T)
    g, r = t // 512, t % 512
    w, q = r // 64, r % 64
    return g * 512 + (w % 2) * 256 + (w // 2) * 64 + q


def kernel(**inputs):
    T = BW * N
    lam = 1.0 / (1.0 + math.exp(-float(inputs["lambda_q1"][0]) * float(inputs["lambda_k1"][0]))) \
        + LAMBDA_INIT
    consts = _prep_consts(inputs, lam)

    if "nc" not in _CACHE:
        _CACHE["nc"] = build_bass(T)
    nc = _CACHE["nc"]

    x = np.asarray(inputs["x"], np.float32)
    dino = np.asarray(inputs["dino_mat"], np.float32)
    pf = np.asarray(inputs["point_feature"], np.float32)
    perm = _tok_perm(T)
    bf = ml_dtypes.bfloat16
    f8 = ml_dtypes.float8_e4m3fn

    in_maps = []
    for c in range(NCORES):
        ws = slice(c * BW, (c + 1) * BW)
        xc = x[ws].reshape(T, C).T                                    # [256, T]
        dc = dino[ws].reshape(T, 1024).T
        pfc = pf[ws].reshape(T, 3).T
        pfT_full = np.concatenate([pfc, np.ones((1, T), np.float32)], 0)
        m = {"xT": np.ascontiguousarray(xc.astype(bf)),
             "dinoT": np.ascontiguousarray(dc.astype(bf)),
             "pfT": np.ascontiguousarray(pfT_full)}
        m.update(consts)
        in_maps.append(m)

    res = run_bass_kernel_spmd(nc, in_maps, list(range(NCORES)), **_CACHE.get("run_kwargs", {}))
    out = np.empty((B, N, C), np.float32)
    for c in range(NCORES):
        oT = res.results[c]["outT"]                                   # [256, T] permuted cols
        out[c * BW:(c + 1) * BW] = oT[:, perm].T.reshape(BW, N, C)
    _CACHE["last_res"] = res
    return out


# revision 68
# speedup vs baseline: 1.1007x; 1.1007x over previous
"""DifferentialWindowAttention TRN2 kernel — 8-core SPMD, data-parallel over windows.

Layout: channel-transposed (CT) activations [C(part), tokens(free)].
 - Projections as CT GEMMs; the heavy ones (dino 1024-dim, q, kg, ks, vtok) run in
   fp8e4m3 with DoubleRow perf mode (two 128-row contraction blocks per instruction
   at 0.5 cyc/col). Power-of-2 weight scaling keeps fp8 in range; the psum->sbuf
   copy descales (scalar activation scale) and adds biases (per-partition bias AP).
 - Attention: S^T[m,q] = (kT-slice as lhsT) @ (qT-slice as rhs); softmax without
   max-subtraction (logits are tiny): U = exp(S^T) * exp_rpb.
   Denominators via PE band-select ones-matmuls -> [4*32-band, (wpair,q)] broadcast.
 - AV: lhsT = token-major V slices, rhs = U slices -> Z^T in CT layout.
 - sub-LN in CT: stats via PE ones-matmuls; rstd = exp(-0.5*ln(var+eps)) on the
   scalar engine (keeps every activation in the natural_log_exp table set -> no
   ACT table reloads); gamma folded into w_proj rows, beta folded into the output
   bias column.
 - DRAM output is [C, T] in a fixed token permutation inverted on host.
"""
import math
import numpy as np
import ml_dtypes

import concourse.bass as bass
import concourse.tile as tile
from concourse import mybir
from concourse.bass_utils import run_bass_kernel_spmd

BF16 = mybir.dt.bfloat16
F32 = mybir.dt.float32
F32R = mybir.dt.float32r
FP8 = mybir.dt.float8e4
AF = mybir.ActivationFunctionType
ALU = mybir.AluOpType
DR = mybir.MatmulPerfMode.DoubleRow

B, N, C, H, D, WIN = 1024, 64, 256, 8, 32, 8
NCORES = 8
BW = B // NCORES            # windows per core
LAMBDA_INIT = 0.8 - 0.6 * math.exp(-0.3 * 1)
EPS = 1e-5

SQ = 256.0   # fp8 scale for wq / wdino (+ident)
SK = 64.0    # fp8 scale for wkvg / wkvs / wkvsn / w2g

_CACHE = {}


def _legalize_waits(nc, max_waits=1):
    """Old walrus in this container allows one sync-wait per instruction;
    hoist extras into standalone EventSemaphore instructions just before."""
    ctr = 0
    for f in nc.m.functions:
        for bb in f.blocks:
            new = []
            for inst in bb.instructions:
                si = inst.sync_info
                if si is not None and si.on_wait and len(si.on_wait) > max_waits:
                    waits = list(si.on_wait)
                    for w in waits[max_waits:]:
                        ctr += 1
                        ev = mybir.InstEventSemaphore(
                            name=f"waitfix_{ctr}", ins=[], outs=[],
                            engine=inst.engine,
                            sync_info=mybir.SyncInfo(on_wait=[w], on_update=[]))
                        new.append(ev)
                    inst.sync_info = mybir.SyncInfo(on_wait=waits[:max_waits],
                                                    on_update=list(si.on_update or []))
                new.append(inst)
            bb.instructions = new
    return ctr


def build_bass(T, tap=None, legalize=True):
    NG = T // 512
    TG = 512
    nc = bass.Bass()
    xT = nc.declare_dram_parameter("xT", [C, T], BF16, isOutput=False)
    dinoT = nc.declare_dram_parameter("dinoT", [1024, T], BF16, isOutput=False)
    pfT = nc.declare_dram_parameter("pfT", [4, T], F32R, isOutput=False)
    bias6_d = nc.declare_dram_parameter("bias6", [C, 6], F32, isOutput=False)
    wq_a = nc.declare_dram_parameter("wq_a", [C, C], BF16, isOutput=False)
    wkvg_a = nc.declare_dram_parameter("wkvg_a", [C, 2 * C], BF16, isOutput=False)
    wdino_a = nc.declare_dram_parameter("wdino_a", [1024, C], BF16, isOutput=False)
    wkvs_a = nc.declare_dram_parameter("wkvs_a", [C, 2 * C], BF16, isOutput=False)
    wkvsn_a = nc.declare_dram_parameter("wkvsn_a", [C, C], BF16, isOutput=False)
    w2g_a = nc.declare_dram_parameter("w2g_a", [4, 2 * C], F32R, isOutput=False)
    wproj_a = nc.declare_dram_parameter("wproj_a", [C, C], F32R, isOutput=False)
    ident_d = nc.declare_dram_parameter("ident", [128, 128], BF16, isOutput=False)
    band_d = nc.declare_dram_parameter("band", [2, 128, 32], BF16, isOutput=False)
    rpb_d = nc.declare_dram_parameter("exp_rpb", [128, H * 256], BF16, isOutput=False)
    crow_f_d = nc.declare_dram_parameter("crow_f", [1, 384], F32R, isOutput=False)
    ccol_f_d = nc.declare_dram_parameter("ccol_f", [128, 1], F32R, isOutput=False)
    ceps_d = nc.declare_dram_parameter("ceps", [1, 1], F32, isOutput=False)
    outT = nc.declare_dram_parameter("outT", [C, T], F32, isOutput=True)

    import contextlib
    with tile.TileContext(nc) as tc, contextlib.ExitStack() as ctx:
        singles = ctx.enter_context(tc.tile_pool(name="singles", bufs=1))
        inp = ctx.enter_context(tc.tile_pool(name="inp", bufs=2))
        acts = ctx.enter_context(tc.tile_pool(name="acts", bufs=2))
        attn = ctx.enter_context(tc.tile_pool(name="attn", bufs=3))
        outs = ctx.enter_context(tc.tile_pool(name="outs", bufs=3))
        psum = ctx.enter_context(tc.tile_pool(name="psum", bufs=1, space="PSUM"))

        # ---------------- constants ----------------
        _cn = [0]

        def cload(src, shape, dt):
            _cn[0] += 1
            t = singles.tile(shape, dt, tag=f"c{_cn[0]}", name=f"c{_cn[0]}")
            nc.sync.dma_start(out=t, in_=src)
            return t

        b6 = [cload(bias6_d[0:128, :], [128, 6], F32),
              cload(bias6_d[128:256, :], [128, 6], F32)]
        wq_t = [cload(wq_a[0:128, :], [128, C], BF16),
                cload(wq_a[128:256, :], [128, C], BF16)]
        wkvg_t = [cload(wkvg_a[0:128, :], [128, 2 * C], BF16),
                  cload(wkvg_a[128:256, :], [128, 2 * C], BF16)]
        wdino_t = [cload(wdino_a[k * 128:(k + 1) * 128, :], [128, C], BF16) for k in range(8)]
        wkvs_t = [cload(wkvs_a[0:128, :], [128, 2 * C], BF16),
                  cload(wkvs_a[128:256, :], [128, 2 * C], BF16)]
        wkvsn_t = [cload(wkvsn_a[0:128, :], [128, C], BF16),
                   cload(wkvsn_a[128:256, :], [128, C], BF16)]
        w2g_t = cload(w2g_a[:, :], [4, 2 * C], F32R)
        wproj_t = [cload(wproj_a[0:128, :], [128, C], F32R),
                   cload(wproj_a[128:256, :], [128, C], F32R)]
        ident_t = cload(ident_d[:, :], [128, 128], BF16)
        band_t = [cload(band_d[p, :, :], [128, 32], BF16) for p in range(2)]
        rpb_t = [cload(rpb_d[:, h * 256:(h + 1) * 256], [128, 256], BF16) for h in range(H)]
        crow = cload(crow_f_d[:, :], [1, 384], F32R)
        ones_bc = crow[:, 256:384]
        oneC_col = cload(ccol_f_d[:, :], [128, 1], F32R)
        eps_t = cload(ceps_d[:, :], [1, 1], F32)

        MM = nc.tensor.matmul
        pg = [0]

        def gemm_ps():
            t = psum.tile([128, TG], F32, tag=f"pga{pg[0] % 2}", name=f"pga{pg[0] % 2}")
            pg[0] += 1
            return t

        def ln_ps():
            return psum.tile([128, TG], F32, tag="pgb", name="pgb")

        def make_half_alloc(tag):
            """Yield [128,256] psum slices, packing pairs into [128,512] banks, 2 banks cycling."""
            state = {"i": 0, "cur": None}

            def alloc():
                i = state["i"]
                if i % 2 == 0:
                    state["cur"] = psum.tile([128, 512], F32, tag=f"{tag}{(i // 2) % 2}",
                                             name=f"{tag}{(i // 2) % 2}")
                state["i"] += 1
                return state["cur"][:, (i % 2) * 256:(i % 2) * 256 + 256]
            return alloc

        # First loop iteration computes wrong on this HW (startup race in the
        # scalar/PE pipeline); re-run group 0 at the end to overwrite its output.
        for g in list(range(NG)) + [0]:
            sl = slice(g * TG, (g + 1) * TG)
            # ---------------- loads ----------------
            xt = [inp.tile([128, TG], BF16, tag=f"xt{i}", name=f"xt{i}") for i in range(2)]
            nc.sync.dma_start(out=xt[0], in_=xT[0:128, sl])
            nc.sync.dma_start(out=xt[1], in_=xT[128:256, sl])
            dt_ = [inp.tile([128, TG], BF16, tag=f"dt{k}", name=f"dt{k}") for k in range(8)]
            for k in range(8):
                nc.sync.dma_start(out=dt_[k], in_=dinoT[k * 128:(k + 1) * 128, sl])
            pft = inp.tile([4, TG], F32R, tag="pft", name="pft")
            nc.sync.dma_start(out=pft, in_=pfT[:, sl])

            # ---------------- q GEMM (fp8 DR) ----------------
            q_sb = [acts.tile([128, TG], BF16, tag=f"q{m}", name=f"q{m}") for m in range(2)]
            for m in range(2):
                ps = gemm_ps()
                c0, c1 = m * 128, (m + 1) * 128
                MM(ps, wq_t[0][:, c0:c1], xt[0], start=True, stop=False)
                MM(ps, wq_t[1][:, c0:c1], xt[1], start=False, stop=True)
                nc.scalar.activation(out=q_sb[m], in_=ps, func=AF.Identity,
                                     bias=b6[m][:, 0:1])

            if tap == "q":
                for m in range(2):
                    nc.gpsimd.dma_start(out=outT[m * 128:(m + 1) * 128, sl], in_=q_sb[m])
                continue
            # ---------------- sem_enh GEMM (fp8 DR dino + scaled-identity x) --------
            se_sb = [acts.tile([128, TG], BF16, tag=f"se{m}", name=f"se{m}") for m in range(2)]
            for m in range(2):
                ps = gemm_ps()
                c0, c1 = m * 128, (m + 1) * 128
                for k in range(8):
                    MM(ps, wdino_t[k][:, c0:c1], dt_[k], start=(k == 0), stop=False)
                MM(ps, ident_t, xt[m], start=False, stop=True)
                nc.scalar.activation(out=se_sb[m], in_=ps, func=AF.Identity,
                                     bias=b6[m][:, 2:3])

            if tap == "se":
                for m in range(2):
                    nc.gpsimd.dma_start(out=outT[m * 128:(m + 1) * 128, sl], in_=se_sb[m])
                continue
            # ---------------- k_geo / k_sem GEMMs (fp8 DR) ----------------
            kg_sb = [acts.tile([128, TG], BF16, tag=f"kg{m}", name=f"kg{m}") for m in range(2)]
            ks_sb = [acts.tile([128, TG], BF16, tag=f"ks{m}", name=f"ks{m}") for m in range(2)]
            for m in range(2):
                c0, c1 = m * 128, (m + 1) * 128
                ps = gemm_ps()
                MM(ps, wkvg_t[0][:, c0:c1], xt[0], start=True, stop=False)
                MM(ps, wkvg_t[1][:, c0:c1], xt[1], start=False, stop=False)
                MM(ps, w2g_t[:, c0:c1], pft, start=False, stop=True)
                nc.scalar.copy(out=kg_sb[m], in_=ps)
            for m in range(2):
                c0, c1 = m * 128, (m + 1) * 128
                ps = gemm_ps()
                MM(ps, wkvs_t[0][:, c0:c1], se_sb[0], start=True, stop=False)
                MM(ps, wkvs_t[1][:, c0:c1], se_sb[1], start=False, stop=True)
                nc.scalar.activation(out=ks_sb[m], in_=ps, func=AF.Identity,
                                     bias=b6[m][:, 1:2])

            if tap in ("kg", "ks"):
                tt_ = {"kg": kg_sb, "ks": ks_sb}[tap]
                for m in range(2):
                    nc.gpsimd.dma_start(out=outT[m * 128:(m + 1) * 128, sl], in_=tt_[m])
                continue
            # ---------------- token-major V GEMMs (fp8 DR) ----------------
            # vtok[kind][c] : [128 tok = 2 windows, 256 = 8h x 32d], c = token chunk
            vtok = {"vm": [], "vs": []}
            for c in range(4):
                t0c = c * 128
                vps = psum.tile([128, 512], F32, tag=f"pu{c % 2}", name=f"pu{c % 2}")
                ps = vps[:, 0:256]
                MM(ps, xt[0][:, t0c:t0c + 128], wkvg_t[0][:, 256:512], start=True, stop=False)
                MM(ps, xt[1][:, t0c:t0c + 128], wkvg_t[1][:, 256:512], start=False, stop=False)
                MM(ps, pft[:, t0c:t0c + 128], w2g_t[:, 256:512], start=False, stop=False)
                MM(ps, se_sb[0][:, t0c:t0c + 128], wkvsn_t[0], start=False, stop=False)
                MM(ps, se_sb[1][:, t0c:t0c + 128], wkvsn_t[1], start=False, stop=True)
                vt = attn.tile([128, 256], BF16, tag=f"vt_vm{c}", name=f"vt_vm{c}")
                nc.scalar.copy(out=vt, in_=ps)
                vtok["vm"].append(vt)
                ps2 = vps[:, 256:512]
                MM(ps2, se_sb[0][:, t0c:t0c + 128], wkvs_t[0][:, 256:512], start=True, stop=False)
                MM(ps2, se_sb[1][:, t0c:t0c + 128], wkvs_t[1][:, 256:512], start=False, stop=True)
                vt2 = attn.tile([128, 256], BF16, tag=f"vt_vs{c}", name=f"vt_vs{c}")
                nc.scalar.copy(out=vt2, in_=ps2)
                vtok["vs"].append(vt2)

            # ---------------- attention ----------------
            # U tiles per (branch, head): [128 = m + 64*(w%2), 256 = (w//2)*64 + q]
            Ur = {}
            qk_ps = make_half_alloc("pu")
            for br, ktiles in (("g", kg_sb), ("s", ks_sb)):
                for h in range(H):
                    kt = ktiles[h // 4]
                    qt = q_sb[h // 4]
                    r0 = (h % 4) * 32
                    ps = qk_ps()
                    for w in range(8):
                        MM(ps[64 * (w % 2):64 * (w % 2) + 64, (w // 2) * 64:(w // 2) * 64 + 64],
                           kt[r0:r0 + 32, w * 64:(w + 1) * 64],
                           qt[r0:r0 + 32, w * 64:(w + 1) * 64],
                           start=True, stop=True,
                           tile_position=(r0, 64 * (w % 2)))
                    ue = attn.tile([128, 256], BF16, tag=f"ue_{br}{h}", name=f"ue_{br}{h}")
                    nc.scalar.activation(out=ue, in_=ps, func=AF.Exp)
                    ur = attn.tile([128, 256], BF16, tag=f"ur_{br}{h}", name=f"ur_{br}{h}")
                    nc.gpsimd.tensor_tensor(out=ur, in0=ue, in1=rpb_t[h], op=ALU.mult)
                    Ur[(br, h)] = ur

            if tap == "U":
                nc.gpsimd.dma_start(out=outT[0:128, slice(g*512, g*512+256)], in_=Ur[("g", 0)])
                nc.gpsimd.dma_start(out=outT[128:256, slice(g*512, g*512+256)], in_=Ur[("g", 1)])
                nc.gpsimd.dma_start(out=outT[0:128, slice(g*512+256, g*512+512)], in_=Ur[("s", 0)])
                nc.gpsimd.dma_start(out=outT[128:256, slice(g*512+256, g*512+512)], in_=Ur[("s", 1)])
                continue
            if tap == "vt":
                for c in range(2):
                    nc.gpsimd.dma_start(out=outT[0:128, slice(g*512+c*256, g*512+(c+1)*256)], in_=vtok["vm"][c])
                    nc.gpsimd.dma_start(out=outT[128:256, slice(g*512+c*256, g*512+(c+1)*256)], in_=vtok["vs"][c])
                continue
            # AV with interleaved denominator band-matmuls: the band MMs for each
            # (q2, par) issue right before its AV block so the reciprocal-gated
            # prs recycling does not head-of-line-block the in-order PE queue.
            rs = {}
            rs_ps = make_half_alloc("prs")
            opre = {}
            for q2 in range(2):
                for par in range(2):
                    for br in ("g", "s"):
                        ps = rs_ps()
                        for hp in range(4):
                            MM(ps[hp * 32:(hp + 1) * 32, :],
                               band_t[par], Ur[(br, 4 * q2 + hp)],
                               start=True, stop=True,
                               tile_position=(0, hp * 32))
                        r = attn.tile([128, 256], F32, tag=f"rs_{br}{q2}{par}", name=f"rs_{br}{q2}{par}")
                        nc.vector.reciprocal(out=r, in_=ps)
                        rs[(br, q2, par)] = r
                    zt = psum.tile([128, 512], F32, tag="pz0", name="pz0")
                    zps = {}
                    for br, kind in (("g", "vm"), ("s", "vs")):
                        ps = zt[:, 0:256] if br == "g" else zt[:, 256:512]
                        for hp in range(4):
                            h = 4 * q2 + hp
                            for wp in range(4):
                                MM(ps[hp * 32:(hp + 1) * 32, wp * 64:(wp + 1) * 64],
                                   vtok[kind][wp][64 * par:64 * par + 64, h * 32:(h + 1) * 32],
                                   Ur[(br, h)][64 * par:64 * par + 64, wp * 64:(wp + 1) * 64],
                                   start=True, stop=True,
                                   tile_position=(64 * par, hp * 32))
                        zps[br] = ps
                    t1 = outs.tile([128, 256], F32, tag="t1", name="t1")
                    t2 = outs.tile([128, 256], F32, tag="t2", name="t2")
                    nc.vector.tensor_tensor(out=t1, in0=zps["g"], in1=rs[("g", q2, par)], op=ALU.mult)
                    nc.vector.tensor_tensor(out=t2, in0=zps["s"], in1=rs[("s", q2, par)], op=ALU.mult)
                    # osq: [:, 0:256] = t1 + t2 + v-bias col, [:, 256:512] = square (stats)
                    op_ = outs.tile([128, 512], F32R, tag=f"opre{q2}{par}", name=f"opre{q2}{par}")
                    nc.vector.scalar_tensor_tensor(out=op_[:, 0:256], in0=t1,
                                                   scalar=b6[q2][:, 3:4], in1=t2,
                                                   op0=ALU.add, op1=ALU.add)
                    nc.gpsimd.tensor_tensor(out=op_[:, 256:512], in0=op_[:, 0:256],
                                            in1=op_[:, 0:256], op=ALU.mult)
                    opre[(q2, par)] = op_

            if tap == "rs":
                for i, k_ in enumerate([("g",0,0),("g",0,1),("g",1,0),("g",1,1)][:2]):
                    nc.sync.dma_start(out=outT[i*128:(i+1)*128, slice(g*512, g*512+256)], in_=rs[k_])
                continue
            if tap == "opre":
                for q2 in range(2):
                    for par in range(2):
                        nc.sync.dma_start(out=outT[q2*128:(q2+1)*128, slice(g*512+par*256, g*512+(par+1)*256)], in_=opre[(q2, par)][:, 0:256])
                continue
            # ---------------- sub-LN (CT) + final projection ----------------
            # gamma folded into wproj rows; beta folded into output bias col.
            for par in range(2):
                stb = ln_ps()
                st = stb[0:1, :]
                MM(st, oneC_col, opre[(0, par)], start=True, stop=False)
                MM(st, oneC_col, opre[(1, par)], start=False, stop=True)
                stmu = outs.tile([1, 256], F32R, tag="stmu", name="stmu")
                nc.scalar.copy(out=stmu, in_=st[0:1, 0:256])
                stsq = outs.tile([1, 256], F32, tag="stsq", name="stsq")
                nc.scalar.copy(out=stsq, in_=st[0:1, 256:512])
                musq = outs.tile([1, 256], F32, tag="musq", name="musq")
                nc.vector.tensor_tensor(out=musq, in0=stmu, in1=stmu, op=ALU.mult)
                var = outs.tile([1, 256], F32, tag="var", name="var")
                nc.vector.tensor_tensor(out=var, in0=stsq, in1=musq, op=ALU.subtract)
                # rstd = exp(-0.5 * ln(var + eps)) — scalar engine, same ACT table set
                lnv = outs.tile([1, 256], F32, tag="lnv", name="lnv")
                nc.scalar.activation(out=lnv, in_=var, func=AF.Ln, bias=eps_t)
                rstd = outs.tile([1, 256], F32R, tag="rstd", name="rstd")
                nc.scalar.activation(out=rstd, in_=lnv, func=AF.Exp, scale=-0.5)
                bc = ln_ps()
                MM(bc[:, 0:256], ones_bc, stmu, start=True, stop=True)
                MM(bc[:, 256:512], ones_bc, rstd, start=True, stop=True)
                ln = [outs.tile([128, 256], F32R, tag=f"ln{q2}", name=f"ln{q2}") for q2 in range(2)]
                for q2 in range(2):
                    d1 = outs.tile([128, 256], F32, tag="d1", name="d1")
                    nc.vector.tensor_tensor(out=d1, in0=opre[(q2, par)][:, 0:256], in1=bc[:, 0:256], op=ALU.subtract)
                    nc.vector.tensor_tensor(out=ln[q2], in0=d1, in1=bc[:, 256:512], op=ALU.mult)
                pj = ln_ps()
                for m in range(2):
                    c0, c1 = m * 128, (m + 1) * 128
                    ps = pj[:, m * 256:(m + 1) * 256]
                    MM(ps, wproj_t[0][:, c0:c1], ln[0], start=True, stop=False)
                    MM(ps, wproj_t[1][:, c0:c1], ln[1], start=False, stop=True)
                    of = outs.tile([128, 256], F32, tag=f"of{m}", name=f"of{m}")
                    nc.scalar.activation(out=of, in_=ps, func=AF.Identity,
                                         bias=b6[m][:, 4:5])
                    nc.sync.dma_start(out=outT[c0:c1, g * TG + par * 256: g * TG + (par + 1) * 256],
                                      in_=of)
    if legalize:
        _legalize_waits(nc)
    return nc


# ====================== host side ======================

def _dr_pack(w, scale):
    """[256, M] f32 -> [128, 2, M] fp8 DoubleRow lhsT (k-blocks side by side)."""
    f8 = ml_dtypes.float8_e4m3fn
    w = w * scale
    return np.stack([w[0:128, :], w[128:256, :]], 1).astype(f8)


def _prep_consts(inputs, lam):
    f = np.float32
    f8 = ml_dtypes.float8_e4m3fn
    wq = inputs["wq"].astype(f) * (D ** -0.5)
    bq = inputs["bq"].astype(f) * (D ** -0.5)
    wkv_geo = inputs["wkv_geo"].astype(f)
    gw = float(inputs["geo_weight"])
    sw = float(inputs["sem_weight"])
    w2g = gw * (inputs["w_geo_proj"].astype(f) @ wkv_geo)             # [3, 512]
    b2g = inputs["bkv_geo"].astype(f) + gw * (inputs["b_geo_proj"].astype(f) @ wkv_geo)
    w2g_a = np.concatenate([w2g, b2g[None, :]], 0)                    # [4, 512]
    wdino = sw * inputs["w_dino_proj"].astype(f)                      # [1024, 256]
    bdino = sw * inputs["b_dino_proj"].astype(f)
    wkv_sem = inputs["wkv_sem"].astype(f)
    bkv_sem = inputs["bkv_sem"].astype(f)
    sc = f(1.0 - LAMBDA_INIT)
    gamma_s = inputs["ln_gamma"].astype(f) * sc
    beta_s = inputs["ln_beta"].astype(f) * sc
    w_proj = inputs["w_proj"].astype(f)
    wproj_a = gamma_s[:, None] * w_proj                               # gamma fold
    bias6 = np.stack([bq, bkv_sem[0:256], bdino,
                      (1.0 - lam) * bkv_sem[256:512],
                      inputs["b_proj"].astype(f) + beta_s @ w_proj,
                      np.zeros(C, f)], 1)                             # [256, 6]
    wkvsn_a = (-lam) * wkv_sem[:, 256:512]           # [256, 256]
    # exp(rpb) transposed, tiled [128, H*256]
    rpb = inputs["rpb_table"].astype(f)[np.asarray(inputs["rp_index"]).reshape(-1)]
    rpb = rpb.reshape(N, N, H)                                        # [n(q), m, H]
    ex = np.exp(rpb.transpose(2, 1, 0))                               # [H, m, q]
    rpb_tiles = np.zeros((128, H * 256), f)
    for h in range(H):
        blk = np.tile(ex[h], (2, 4)).reshape(128, 256)                # [m+64wp, wpair*64+q]
        rpb_tiles[:, h * 256:(h + 1) * 256] = blk
    ident = np.eye(128, dtype=f)
    band = np.zeros((2, 128, 32), f)
    band[0, 0:64, :] = 1.0
    band[1, 64:128, :] = 1.0
    bf = ml_dtypes.bfloat16
    return {
        "bias6": bias6, "wq_a": wq.astype(bf), "wkvg_a": wkv_geo.astype(bf),
        "wdino_a": wdino.astype(bf), "wkvs_a": wkv_sem.astype(bf),
        "wkvsn_a": wkvsn_a.astype(bf),
        "w2g_a": w2g_a, "wproj_a": wproj_a,
        "ident": ident.astype(bf), "band": band.astype(bf),
        "exp_rpb": rpb_tiles.astype(bf),
        "crow_f": np.ones((1, 384), f),
        "ccol_f": np.full((128, 1), 1.0 / C, f), "ceps": np.full((1, 1), EPS, f),
    }


def _tok_perm(T):
    # device column for linear token t (within a core)
    t = np.arange(T)
    g, r = t // 512, t % 512
    w, q = r // 64, r % 64
    return g * 512 + (w % 2) * 256 + (w // 2) * 64 + q


def kernel(**inputs):
    T = BW * N
    lam = 1.0 / (1.0 + math.exp(-float(inputs["lambda_q1"][0]) * float(inputs["lambda_k1"][0]))) \
        + LAMBDA_INIT
    consts = _prep_consts(inputs, lam)

    if "nc" not in _CACHE:
        _CACHE["nc"] = build_bass(T)
    nc = _CACHE["nc"]

    x = np.asarray(inputs["x"], np.float32)
    dino = np.asarray(inputs["dino_mat"], np.float32)
    pf = np.asarray(inputs["point_feature"], np.float32)
    perm = _tok_perm(T)
    bf = ml_dtypes.bfloat16
    f8 = ml_dtypes.float8_e4m3fn

    in_maps = []
    for c in range(NCORES):
        ws = slice(c * BW, (c + 1) * BW)
        xc = x[ws].reshape(T, C).T                                    # [256, T]
        dc = dino[ws].reshape(T, 1024).T
        pfc = pf[ws].reshape(T, 3).T
        pfT_full = np.concatenate([pfc, np.ones((1, T), np.float32)], 0)
        m = {"xT": np.ascontiguousarray(xc.astype(bf)),
             "dinoT": np.ascontiguousarray(dc.astype(bf)),
             "pfT": np.ascontiguousarray(pfT_full)}
        m.update(consts)
        in_maps.append(m)

    res = run_bass_kernel_spmd(nc, in_maps, list(range(NCORES)), **_CACHE.get("run_kwargs", {}))
    out = np.empty((B, N, C), np.float32)
    for c in range(NCORES):
        oT = res.results[c]["outT"]                                   # [256, T] permuted cols
        out[c * BW:(c + 1) * BW] = oT[:, perm].T.reshape(BW, N, C)
    _CACHE["last_res"] = res
    return out
